# revision 14
# baseline (speedup 1.0000x reference)
"""MoE layer (nn_MoELayer_4681514353281) Trainium2 Bass kernel.

Reference semantics: for slot i in range(4), expert i's FFN (W1 + A1@B1 LoRA,
gelu-tanh, W2 + A2@B2 LoRA) runs densely over ALL tokens; per-token combine
weight = renormalized top-4 softmax gate weight where top_idx == i (else 0).
Only experts 0-3 are ever used.

Sharding: 8 cores = 4 experts x 2 halves of F (expert-parallel + intra-expert
F-split). Each core computes w_e[t] * (partial expert output for its F-half)
for all 8192 tokens; host sums the 8 partial [8192, 1024] outputs.

The gate's top-4 selection needs ~1e-6 logit precision to reproduce the fp32
reference's picks (near-ties flip otherwise), which exceeds the PE's FP22
multiply path; the 8192x16 softmax/top-4 (0.13% of FLOPs) is computed on the
host and shipped as a [128, 64] weight table. All FFN compute (99.87% of
FLOPs) runs on device in float32r at full PE rate.
"""

import os
import sys

sys.path.insert(0, "/opt/trn_rl_repo")

import numpy as np

# Problem dims (hardcoded per spec)
B, S, D, F, E, R = 2, 4096, 1024, 4096, 16, 16
TOPK = 4
N_TOK = B * S          # 8192
F2 = F // 2            # 2048 per-core F half
TOK_BLK = 256
DC = D // 128          # 8
FC = F2 // 128         # 16

_programs = {}
LAST_RESULTS = None


def _build_program(n_blk):
    import concourse.tile as tile
    from concourse import bacc, mybir

    F32R = mybir.dt.float32r
    F32 = mybir.dt.float32
    AF = mybir.ActivationFunctionType

    nc = bacc.Bacc("TRN2", target_bir_lowering=False, debug=False, num_devices=8)

    xTd = nc.dram_tensor("xT", [D, N_TOK], F32R, kind="ExternalInput")
    w1d = nc.dram_tensor("w1", [D, F2], F32R, kind="ExternalInput")
    a1d = nc.dram_tensor("a1", [D, R], F32R, kind="ExternalInput")
    b1d = nc.dram_tensor("b1", [R, F2], F32R, kind="ExternalInput")
    w2d = nc.dram_tensor("w2", [F2, D], F32R, kind="ExternalInput")
    a2d = nc.dram_tensor("a2", [F2, R], F32R, kind="ExternalInput")
    b2d = nc.dram_tensor("b2", [R, D], F32R, kind="ExternalInput")
    wcd = nc.dram_tensor("wc", [128, N_TOK // 128], F32, kind="ExternalInput")
    outd = nc.dram_tensor("out", [N_TOK, D], F32, kind="ExternalOutput")

    with tile.TileContext(nc) as tc:
        with (
            tc.tile_pool(name="singles", bufs=1) as singles,
            tc.tile_pool(name="xp", bufs=2) as xp,
            tc.tile_pool(name="hp", bufs=3) as hp,
            tc.tile_pool(name="smallp", bufs=2) as smallp,
            tc.tile_pool(name="outp", bufs=3) as outp,
            tc.tile_pool(name="psH", bufs=2, space="PSUM") as psH,
            tc.tile_pool(name="psEO", bufs=2, space="PSUM") as psEO,
            tc.tile_pool(name="psU2", bufs=1, space="PSUM") as psU2,
            tc.tile_pool(name="psU1", bufs=1, space="PSUM") as psU1,
        ):
            # ---- resident weights ----
            w1 = singles.tile([128, FC, DC, 128], F32R)   # [p, fc, dc, q]
            w2 = singles.tile([128, FC, D], F32R)         # [p, fc, d]
            a1 = singles.tile([128, DC, R], F32R)
            a2 = singles.tile([128, FC, R], F32R)
            b1 = singles.tile([R, F2], F32R)
            b2 = singles.tile([R, D], F32R)
            w_all = singles.tile([128, N_TOK // 128], F32)

            xT_r = xTd.rearrange("(dc p) t -> p dc t", p=128)
            w1_r = w1d.rearrange("(dc p) (fc q) -> p fc dc q", p=128, q=128)
            w2_r = w2d.rearrange("(fc p) d -> p fc d", p=128)

            def load_block(b):
                t = xp.tile([128, DC, TOK_BLK], F32R, tag="xb")
                nc.scalar.dma_start(
                    t[:], xT_r[:, :, b * TOK_BLK:(b + 1) * TOK_BLK]
                )
                return t

            xb = load_block(0)

            # small residents first, then per-fc w1/w2 interleaved so early
            # f-chunks arrive before their first consumers
            nc.sync.dma_start(a1[:], a1d.rearrange("(dc p) r -> p dc r", p=128))
            nc.sync.dma_start(b1[:], b1d[:, :])
            nc.sync.dma_start(w_all[:], wcd[:, :])
            nc.sync.dma_start(a2[:], a2d.rearrange("(fc p) r -> p fc r", p=128))
            for fc in range(FC):
                nc.sync.dma_start(w1[:, fc, :, :], w1_r[:, fc, :, :])
                nc.sync.dma_start(w2[:, fc, :], w2_r[:, fc, :])
            nc.sync.dma_start(b2[:], b2d[:, :])

            for blk in range(n_blk):
                # LoRA up: u1 = x^T A1  -> [R, TOK_BLK]
                ps_u1 = psU1.tile([R, TOK_BLK], F32)
                for dc in range(DC):
                    nc.tensor.matmul(
                        ps_u1[:], a1[:, dc, :], xb[:, dc, :],
                        start=(dc == 0), stop=(dc == DC - 1),
                    )
                u1 = smallp.tile([R, TOK_BLK], F32R, tag="u1")
                nc.vector.tensor_copy(u1[:], ps_u1[:])

                ps_u2 = psU2.tile([R, TOK_BLK], F32)
                ps_eo0 = psEO.tile([128, D], F32, tag="ps_eo")
                ps_eo1 = psEO.tile([128, D], F32, tag="ps_eo")
                ps_eo = [ps_eo0, ps_eo1]

                def tail(fcp, hprev):
                    nc.tensor.matmul(
                        ps_u2[:], a2[:, fcp, :], hprev[:],
                        start=(fcp == 0), stop=(fcp == FC - 1),
                    )
                    for sub in range(2):
                        for dh in range(2):
                            nc.tensor.matmul(
                                ps_eo[sub][:, dh * 512:(dh + 1) * 512],
                                hprev[:, sub * 128:(sub + 1) * 128],
                                w2[:, fcp, dh * 512:(dh + 1) * 512],
                                start=(fcp == 0), stop=False,
                            )

                h_prev = None
                for fc in range(FC):
                    ps_h = psH.tile([128, TOK_BLK], F32)
                    for dc in range(DC):
                        nc.tensor.matmul(
                            ps_h[:], w1[:, fc, dc, :], xb[:, dc, :],
                            start=(dc == 0), stop=False,
                        )
                    nc.tensor.matmul(
                        ps_h[:], b1[:, fc * 128:(fc + 1) * 128], u1[:],
                        start=False, stop=True,
                    )
                    h = hp.tile([128, TOK_BLK], F32R, tag="h")
                    nc.scalar.activation(h[:], ps_h[:], AF.Gelu_apprx_tanh)
                    if h_prev is not None:
                        tail(fc - 1, h_prev)
                    h_prev = h
                tail(FC - 1, h_prev)

                u2 = smallp.tile([R, TOK_BLK], F32R, tag="u2")
                nc.vector.tensor_copy(u2[:], ps_u2[:])
                for sub in range(2):
                    for dh in range(2):
                        nc.tensor.matmul(
                            ps_eo[sub][:, dh * 512:(dh + 1) * 512],
                            u2[:, sub * 128:(sub + 1) * 128],
                            b2[:, dh * 512:(dh + 1) * 512],
                            start=False, stop=True,
                        )

                # prefetch next block's x while we drain
                if blk + 1 < n_blk:
                    xb_next = load_block(blk + 1)
                else:
                    xb_next = None

                for sub in range(2):
                    ob = outp.tile([128, D], F32, tag="ob")
                    col = 2 * blk + sub
                    nc.vector.tensor_scalar_mul(
                        ob[:], ps_eo[sub][:], scalar1=w_all[:, col:col + 1]
                    )
                    t0 = blk * TOK_BLK + sub * 128
                    nc.scalar.dma_start(outd[t0:t0 + 128, :], ob[:])

                xb = xb_next

    nc.compile()
    return nc


def _get_program(n_blk):
    if n_blk not in _programs:
        _programs[n_blk] = _build_program(n_blk)
    return _programs[n_blk]


def _gate_weights(x2d, Wg):
    """Reference-faithful gate: fp32 softmax, top-4 (lowest index on ties),
    renormalize. Returns [N_TOK, 4] combine weights for experts 0-3."""
    logits = x2d.astype(np.float32) @ Wg.astype(np.float32)
    m = logits.max(axis=-1, keepdims=True)
    e = np.exp((logits - m).astype(np.float32), dtype=np.float32)
    p = (e / e.sum(axis=-1, keepdims=True).astype(np.float32)).astype(np.float32)
    idx = np.argsort(-p, axis=-1, kind="stable")[:, :TOPK]
    topw = np.take_along_axis(p, idx, axis=-1)
    topw = (topw / topw.sum(axis=-1, keepdims=True)).astype(np.float32)
    w = np.zeros((x2d.shape[0], TOPK), np.float32)
    for i in range(TOPK):
        w[:, i] = (topw * (idx == i)).sum(axis=-1)
    return w


def kernel(x, Wg, W1, A1, B1, W2, A2, B2):
    global LAST_RESULTS
    from concourse.bass_utils import run_bass_kernel_spmd

    n_blk = int(os.environ.get("KERNEL_NBLK", N_TOK // TOK_BLK))
    nc = _get_program(n_blk)

    x = np.asarray(x, dtype=np.float32)
    x2d = x.reshape(N_TOK, D)
    xT = np.ascontiguousarray(x2d.T)
    w4 = _gate_weights(x2d, np.asarray(Wg, dtype=np.float32))

    in_maps = []
    for core in range(8):
        e = core % 4
        half = core // 4
        f0, f1 = half * F2, (half + 1) * F2
        # [128, N_TOK//128]: column c holds tokens [c*128, (c+1)*128)
        wc = np.ascontiguousarray(w4[:, e].reshape(N_TOK // 128, 128).T)
        in_maps.append({
            "xT": xT,
            "w1": np.ascontiguousarray(W1[e][:, f0:f1], dtype=np.float32),
            "a1": np.ascontiguousarray(A1[e], dtype=np.float32),
            "b1": np.ascontiguousarray(B1[e][:, f0:f1], dtype=np.float32),
            "w2": np.ascontiguousarray(W2[e][f0:f1, :], dtype=np.float32),
            "a2": np.ascontiguousarray(A2[e][f0:f1, :], dtype=np.float32),
            "b2": np.ascontiguousarray(B2[e], dtype=np.float32),
            "wc": wc,
        })

    trace = bool(os.environ.get("KERNEL_TRACE"))
    res = run_bass_kernel_spmd(
        nc, in_maps, core_ids=list(range(8)), trace=trace
    )
    LAST_RESULTS = res

    out = res.results[0]["out"].astype(np.float64)
    for core in range(1, 8):
        out += res.results[core]["out"]
    return out.astype(np.float32).reshape(B, S, D)
